# revision 1
# baseline (speedup 1.0000x reference)
"""Trainium2 Bass kernel for 2-layer GAT (nn_GAT_5970004541990).

Sharding: nodes padded 50000 -> 50176, sharded by destination across 8
NeuronCores (6272 nodes = 49 tiles of 128 each). Self-loops added. Host does
index preprocessing only; all FLOPs run on device.

Per core:
- Phase 1 (replicated): node table h_aug = x @ [W1 | W1 a1s | W1 a1d] + ones
  column, written to DRAM split at row 32768 (dma_gather int16 indices are
  signed). Row stride 192 f32 (768B, 256B-aligned for dma_gather).
- Phase 2 (layer 1): per super-tile of 4 dst tiles, dma_gather all edge-chunk
  source rows; one-hot M = (dstoff == iota); p = exp(leaky_relu(als + ald));
  per 128-edge chunk a PE matmul U += (M*p)^T @ [feat|...|ones] accumulates
  numerator and denominator in PSUM; out1 = relu(U/s); layer-2 table shard
  h2_aug = out1 @ [W2 | W2 a2s | W2 a2d] via PE transpose + matmul.
- Phase 3: AllGather compact layer-2 shards (50176 x 19 f32), expand into
  256B-row tables.
- Phase 4 (layer 2): same aggregation, no relu; per-tile pooling matmul
  accumulates [64, 16] graph sums in PSUM.
- Phase 5: AllReduce pooled sums; mean via host 1/count; log_softmax.
All cores emit the identical [64, 16] output; kernel() returns core 0's.
"""
import sys

sys.path.insert(0, "/opt/trn_rl_repo")

import numpy as np

import concourse.bass as bass
import concourse.mybir as mybir
import concourse.tile as tile
from concourse.tile_rust import add_dep_helper
from concourse import bacc
from concourse.bass_utils import run_bass_kernel_spmd
from concourse.masks import make_identity

# ---------------------------------------------------------------- constants
N_NODES = 50000
N_GRAPHS = 64
D_HID = 128
D_OUT = 16
NEG = 0.2

P = 128
NCORES = 8
NPAD = 50176                     # 8 * 49 * 128
NODES_PER_CORE = NPAD // NCORES  # 6272
TILES_PER_CORE = NODES_PER_CORE // P  # 49
GLOBAL_TILES = NPAD // P         # 392
NLO = 32768                      # table split (signed int16 index limit)
NHI = NPAD - NLO
LO_TILES = NLO // P              # 256
NAD = NPAD // 2                  # 25088 ald-table rows (pair-packed, < 32768)
AD_TILES = NAD // P              # 196

S_TILES = 2                      # dst tiles per super-tile
BF16_T1 = False                  # layer-1 table in bf16 (512B rows)
F32R = False                     # chunk matmuls in fp32r (fast PE fp32)
QSPREAD = 0                      # spread gathers over SWDGE queues
G_BUFS = 5                       # aggregation pool depth
U2_BUFS = 4                      # layer-2 U psum bufs
MP_BUFS = 12
XB_P1 = 8                        # phase-1 tiles per batched DMA
P1PS_BUFS = 4
U1_BUFS = 4
MP_ALT = False
ELEM1 = 192                      # layer-1 table row f32 (768B, %256 == 0)
ELEM1B = 256                     # layer-1 table row bf16 (512B)
ELEM2 = 64                       # layer-2 table row f32 (256B)
ROW1 = 131                       # meaningful cols: feat 0..127, als, ald, ones
ROW2 = 19                        # feat 0..15, als2, ald2, ones

f32 = mybir.dt.float32
i16 = mybir.dt.int16
bf16 = mybir.dt.bfloat16


def _ceil(a, b):
    return -(-a // b)


def _wrap_idx(flat):
    """dma_gather index layout: slot i reads wrapped[i%16, i//16]; the 16-row
    pattern is replicated to all 128 partitions (one per Q7 core)."""
    v16 = flat.astype(np.uint16).view(np.int16)
    return np.tile(v16.reshape(-1, 16).T, (8, 1))


def _pad_to(a, n, fill):
    out = np.full(n, fill, a.dtype)
    out[:len(a)] = a
    return out


# ------------------------------------------------------------ host indexing
def preprocess(edge_index, batch):
    src = np.concatenate(
        [edge_index[0].astype(np.int64), np.arange(NPAD, dtype=np.int64)])
    dst = np.concatenate(
        [edge_index[1].astype(np.int64), np.arange(NPAD, dtype=np.int64)])
    core = dst // NODES_PER_CORE

    # per-core per-tile lo/hi edge lists (order within a tile is arbitrary
    # but src and dstoff must stay aligned)
    cores = []
    nlo = np.zeros((NCORES, TILES_PER_CORE), np.int64)
    nhi = np.zeros((NCORES, TILES_PER_CORE), np.int64)
    for k in range(NCORES):
        m = core == k
        s_k = src[m]
        d_k = dst[m] - k * NODES_PER_CORE
        t_k = d_k // P
        o_k = (d_k % P).astype(np.int16)
        h_k = s_k >= NLO
        lo, hi = {}, {}
        for t in range(TILES_PER_CORE):
            ml = (t_k == t) & ~h_k
            mh = (t_k == t) & h_k
            lo[t] = (s_k[ml], o_k[ml])
            hi[t] = (s_k[mh] - NLO, o_k[mh])
            nlo[k, t] = ml.sum()
            nhi[k, t] = mh.sum()
        cores.append((lo, hi))

    CLO = np.maximum(_ceil(nlo, P).max(axis=0), 1)
    CHI = _ceil(nhi, P).max(axis=0)

    # static chunk schedule (identical on every core)
    supers = []
    chunk_tiles, chunk_start, chunk_stop = [], [], []
    lo_c = hi_c = 0
    for s0 in range(0, TILES_PER_CORE, S_TILES):
        tiles = list(range(s0, min(s0 + S_TILES, TILES_PER_CORE)))
        chunks = [(t, 0) for t in tiles for _ in range(int(CLO[t]))]
        n_lo = len(chunks)
        chunks += [(t, 1) for t in tiles for _ in range(int(CHI[t]))]
        n_hi = len(chunks) - n_lo
        first, last = {}, {}
        for ci, (t, _) in enumerate(chunks):
            first.setdefault(t, ci)
            last[t] = ci
        base = len(chunk_tiles)
        supers.append(dict(tiles=tiles, chunks=chunks, n_lo=n_lo, n_hi=n_hi,
                           base=base, lo_col0=lo_c, hi_col0=hi_c))
        lo_c += n_lo * 8
        hi_c += n_hi * 8
        for ci, (t, _) in enumerate(chunks):
            chunk_tiles.append(t)
            chunk_start.append(ci == first[t])
            chunk_stop.append(ci == last[t])
    TC = len(chunk_tiles)

    # per-core index tensors
    batch_pad = np.full(NPAD, 127, np.int64)
    batch_pad[:N_NODES] = batch.astype(np.int64)
    per_core = []
    for k in range(NCORES):
        lo, hi = cores[k]
        dstoff = np.full((TC, P), -1, np.int16)
        lo_cols, hi_cols = [], []
        for s in supers:
            lo_flat, hi_flat = [], []
            used = {}
            for t in s["tiles"]:
                lo_flat.append(_pad_to(lo[t][0], int(CLO[t]) * P, 0))
                if CHI[t]:
                    hi_flat.append(_pad_to(hi[t][0], int(CHI[t]) * P, 0))
            for ci, (t, h) in enumerate(s["chunks"]):
                row = used.get((t, h), 0)
                used[(t, h)] = row + 1
                offs = (lo if h == 0 else hi)[t][1]
                want = (int(CLO[t]) if h == 0 else int(CHI[t])) * P
                o = _pad_to(offs, want, np.int16(-1))
                dstoff[s["base"] + ci] = o[row * P:(row + 1) * P]
            lo_cols.append(_wrap_idx(np.concatenate(lo_flat)))
            if hi_flat:
                hi_cols.append(_wrap_idx(np.concatenate(hi_flat)))
        idx_lo = np.concatenate(lo_cols, axis=1)
        idx_hi = (np.concatenate(hi_cols, axis=1) if hi_cols
                  else np.zeros((P, 1), np.int16))
        # ald gather: global dst node per slot, pair-packed table
        tile_of_chunk = np.array([t for s in supers for (t, _) in s["chunks"]])
        gnode = (k * NODES_PER_CORE + tile_of_chunk[:, None] * P
                 + np.maximum(dstoff, 0).astype(np.int64))      # [TC, P]
        ad_row = gnode % NAD
        ad_mask = (gnode >= NAD).astype(np.float32)
        idx_ad = _wrap_idx(ad_row.reshape(-1))
        boff = batch_pad[k * NODES_PER_CORE:(k + 1) * NODES_PER_CORE]
        per_core.append(dict(
            idx_lo=np.ascontiguousarray(idx_lo),
            idx_hi=np.ascontiguousarray(idx_hi),
            idx_ad=np.ascontiguousarray(idx_ad),
            admask=np.ascontiguousarray(ad_mask.T),
            dstoff=np.ascontiguousarray(dstoff.T),
            batchoff=np.ascontiguousarray(
                boff.reshape(TILES_PER_CORE, P).T.astype(np.int16))))

    meta = dict(supers=supers, CLO=CLO, CHI=CHI, TC=TC,
                chunk_start=chunk_start, chunk_stop=chunk_stop,
                idx_lo_cols=per_core[0]["idx_lo"].shape[1],
                idx_hi_cols=per_core[0]["idx_hi"].shape[1])
    return meta, per_core


# ------------------------------------------------------------- bass program
def build_program(meta, with_bias1, with_bias2, repeats=None, ablate=(),
                  bf16_t1=None):
    nc = bacc.Bacc("TRN2", target_bir_lowering=False, debug=False,
                   num_devices=NCORES)
    rep = dict(p1=1, l1=1, l2=1)
    rep.update(repeats or {})
    if bf16_t1 is None:
        bf16_t1 = BF16_T1
    t1_dt = bf16 if bf16_t1 else f32
    elem1 = ELEM1B if bf16_t1 else ELEM1
    TC = meta["TC"]
    supers = meta["supers"]
    AF = mybir.ActivationFunctionType
    OP = mybir.AluOpType
    core_ids = list(range(NCORES))

    xT_d = nc.dram_tensor("xT", [P, NPAD], f32, kind="ExternalInput")
    w1_d = nc.dram_tensor("w1aug", [P, 130], f32, kind="ExternalInput")
    w2_d = nc.dram_tensor("w2aug", [P, 18], f32, kind="ExternalInput")
    ilo_d = nc.dram_tensor("idx_lo", [P, meta["idx_lo_cols"]], i16,
                           kind="ExternalInput")
    ihi_d = nc.dram_tensor("idx_hi", [P, meta["idx_hi_cols"]], i16,
                           kind="ExternalInput")
    doff_d = nc.dram_tensor("dstoff", [P, TC], i16, kind="ExternalInput")
    iad_d = nc.dram_tensor("idx_ad", [P, TC * 8], i16, kind="ExternalInput")
    admask_d = nc.dram_tensor("admask", [P, TC], f32, kind="ExternalInput")
    boff_d = nc.dram_tensor("batchoff", [P, TILES_PER_CORE], i16,
                            kind="ExternalInput")
    iota_d = nc.dram_tensor("iota", [P, P], i16, kind="ExternalInput")
    rcnt_d = nc.dram_tensor("recip_cnt", [N_GRAPHS, 1], f32,
                            kind="ExternalInput")
    if with_bias1:
        b1_d = nc.dram_tensor("b1b", [P, D_HID], f32, kind="ExternalInput")
    if with_bias2:
        b2_d = nc.dram_tensor("b2b", [P, D_OUT], f32, kind="ExternalInput")
    out_d = nc.dram_tensor("out", [N_GRAPHS, D_OUT], f32,
                           kind="ExternalOutput")

    t1lo = nc.dram_tensor("t1lo", [NLO, elem1], t1_dt)
    t1hi = nc.dram_tensor("t1hi", [NHI, elem1], t1_dt)
    t1ad = nc.dram_tensor("t1ad", [NAD, ELEM2], f32)
    t2ad = nc.dram_tensor("t2ad", [NAD, ELEM2], f32)
    ag_in = nc.dram_tensor("ag_in", [NODES_PER_CORE, ROW2], f32)
    ag_out = nc.dram_tensor("ag_out", [NPAD, ROW2], f32, addr_space="Shared")
    t2lo = nc.dram_tensor("t2lo", [NLO, ELEM2], f32)
    t2hi = nc.dram_tensor("t2hi", [NHI, ELEM2], f32)
    ar_in = nc.dram_tensor("ar_in", [N_GRAPHS, D_OUT], f32)
    ar_out = nc.dram_tensor("ar_out", [N_GRAPHS, D_OUT], f32,
                            addr_space="Shared")

    def dep(after, *before):
        for b in before:
            add_dep_helper(after.ins, b.ins, reason="phase order")

    with tile.TileContext(nc) as tc:
        with tc.tile_pool(name="res", bufs=1) as res:
            iota_t = res.tile([P, P], i16)
            nc.sync.dma_start(out=iota_t[:], in_=iota_d.ap())
            doff_t = res.tile([P, TC], i16)
            nc.sync.dma_start(out=doff_t[:], in_=doff_d.ap())
            iad_t = res.tile([P, TC * 8], i16)
            nc.sync.dma_start(out=iad_t[:], in_=iad_d.ap())
            admask_t = res.tile([P, TC], f32)
            nc.sync.dma_start(out=admask_t[:], in_=admask_d.ap())
            ilo_t = res.tile([P, meta["idx_lo_cols"]], i16)
            nc.sync.dma_start(out=ilo_t[:], in_=ilo_d.ap())
            ihi_t = res.tile([P, meta["idx_hi_cols"]], i16)
            nc.sync.dma_start(out=ihi_t[:], in_=ihi_d.ap())
            boff_t = res.tile([P, TILES_PER_CORE], i16)
            nc.sync.dma_start(out=boff_t[:], in_=boff_d.ap())
            w1_t = res.tile([P, 130], f32)
            nc.sync.dma_start(out=w1_t[:], in_=w1_d.ap())
            w2_t = res.tile([P, 18], f32)
            nc.sync.dma_start(out=w2_t[:], in_=w2_d.ap())
            rcnt_t = res.tile([N_GRAPHS, 1], f32)
            nc.sync.dma_start(out=rcnt_t[:], in_=rcnt_d.ap())
            ident_t = res.tile([P, P], f32)
            make_identity(nc, ident_t[:])
            if with_bias1:
                b1_t = res.tile([P, D_HID], f32)
                nc.sync.dma_start(out=b1_t[:], in_=b1_d.ap())
            if with_bias2:
                b2_t = res.tile([P, D_OUT], f32)
                nc.sync.dma_start(out=b2_t[:], in_=b2_d.ap())

            # ---------------- phase 1: node table (replicated) -------------
            tab_writes = []
            XB = XB_P1
            with (
                tc.tile_pool(name="p1", bufs=3) as p1,
                tc.tile_pool(name="p1ps", bufs=P1PS_BUFS, space="PSUM") as p1ps,
            ):
              for _r in range(rep["p1"]):
                for gb in range(0, GLOBAL_TILES, XB):
                    nb = min(XB, GLOBAL_TILES - gb)
                    xt = p1.tile([P, XB * P], f32, tag="xt")
                    nc.sync.dma_start(
                        out=xt[:, 0:nb * P],
                        in_=xT_d.ap()[:, gb * P:(gb + nb) * P])
                    stg = p1.tile([P, XB, ROW1], t1_dt, tag="stg")
                    for j in range(nb):
                        gt = gb + j
                        hps = p1ps.tile([P, 130], f32, tag="hps")
                        nc.tensor.matmul(hps[:], xt[:, j * P:(j + 1) * P],
                                         w1_t[:], start=True, stop=True)
                        nc.vector.tensor_copy(stg[:, j, 0:130], hps[:])
                    nc.vector.memset(stg[:, 0:nb, 130:131], 1.0)
                    # one DMA for nb tiles: DRAM rows (j*128+p) <-> src (p, j)
                    if gb < LO_TILES:
                        dst = t1lo.ap()[gb * P:(gb + nb) * P, 0:ROW1]
                    else:
                        r0 = (gb - LO_TILES) * P
                        dst = t1hi.ap()[r0:r0 + nb * P, 0:ROW1]
                    dst = dst.rearrange("(j p) c -> p j c", p=P)
                    tab_writes.append(
                        nc.sync.dma_start(out=dst, in_=stg[:, 0:nb, :]))
                # ald column table from the fat tables (3 strided copies,
                # ordered after every table write)
                fence0 = nc.sync.nop(nofuse=True, hint="fence_p1w")
                dep(fence0, *tab_writes)
                with nc.allow_non_contiguous_dma(
                        reason="4B/row ald column build"):
                    c129 = 129
                    for ad_dma in (
                        nc.sync.dma_start(
                            out=t1ad.ap()[:, 0:1],
                            in_=t1lo.ap()[0:NAD, c129:c129 + 1]),
                        nc.sync.dma_start(
                            out=t1ad.ap()[0:NLO - NAD, 1:2],
                            in_=t1lo.ap()[NAD:NLO, c129:c129 + 1]),
                        nc.sync.dma_start(
                            out=t1ad.ap()[NLO - NAD:NAD, 1:2],
                            in_=t1hi.ap()[:, c129:c129 + 1]),
                    ):
                        dep(ad_dma, fence0)
                        tab_writes.append(ad_dma)

            fence1 = nc.sync.nop(nofuse=True, hint="fence_p1")
            dep(fence1, *tab_writes)   # tab_writes includes the 3 ad builds

            # --------------- shared aggregation loop ----------------------
            def aggregation_layer(layer, pool, psum_u, misc):
                """misc: dict with layer-specific psum pools / tiles."""
                if layer == 1:
                    elem, row, als_c, ones_c = elem1, ROW1, 128, 130
                    tlo, thi, tad = t1lo, t1hi, t1ad
                    g_dt = t1_dt
                else:
                    elem, row, als_c, ones_c = ELEM2, ROW2, 16, 18
                    tlo, thi, tad = t2lo, t2hi, t2ad
                    g_dt = f32
                gathers = []
                ad_gathers = []
                side_writes = []

                for s in supers:
                    sc = len(s["chunks"])
                    n_lo, n_hi, base = s["n_lo"], s["n_hi"], s["base"]
                    G = pool.tile([P, sc, elem], g_dt, tag="G")
                    if "gathers" in ablate:
                        nc.vector.memset(G[:, :, 0:1], 1.0)
                    else:
                     g1 = nc.gpsimd.dma_gather(
                        out_ap=G[:, 0:n_lo, :], in_ap=tlo.ap(),
                        idxs_ap=ilo_t[:, s["lo_col0"]:s["lo_col0"] + n_lo * 8],
                        num_idxs=n_lo * P, num_idxs_reg=n_lo * P,
                        elem_size=elem, single_packet=False)
                     gathers.append(g1)
                     if n_hi:
                        g2 = nc.gpsimd.dma_gather(
                            out_ap=G[:, n_lo:sc, :], in_ap=thi.ap(),
                            idxs_ap=ihi_t[:, s["hi_col0"]:
                                          s["hi_col0"] + n_hi * 8],
                            num_idxs=n_hi * P, num_idxs_reg=n_hi * P,
                            elem_size=elem, single_packet=False,
                            queue_num=(2 if QSPREAD else 0))
                        gathers.append(g2)

                    AD = misc["pool2"].tile([P, sc, ELEM2], f32, tag="AD")
                    if "aldgather" in ablate or "gathers" in ablate:
                        nc.vector.memset(AD[:, :, 0:2], 0.5)
                    else:
                        g3 = nc.gpsimd.dma_gather(
                            out_ap=AD[:], in_ap=tad.ap(),
                            idxs_ap=iad_t[:, base * 8:(base + sc) * 8],
                            num_idxs=sc * P, num_idxs_reg=sc * P,
                            elem_size=ELEM2, single_packet=False,
                            queue_num=(1 if QSPREAD else 0))
                        ad_gathers.append(g3)

                    M = misc["pool2"].tile([P, sc, P], bf16, tag="M")
                    if "mbuild" in ablate:
                        nc.vector.memset(M[:, :, 0:1], 1.0)
                    if "mbuild" not in ablate:
                     nc.vector.tensor_tensor(
                        out=M[:],
                        in0=doff_t[:, base:base + sc].unsqueeze(2)
                            .broadcast_to([P, sc, P]),
                        in1=iota_t[:].unsqueeze(1).broadcast_to([P, sc, P]),
                        op=OP.is_equal)

                    # ald[dst] = AD0 + mask * (AD1 - AD0); e = als[src] + ald
                    adt = pool.tile([P, sc], f32, tag="adt")
                    nc.vector.tensor_sub(adt[:], AD[:, :, 1], AD[:, :, 0])
                    nc.vector.tensor_mul(adt[:], adt[:],
                                         admask_t[:, base:base + sc])
                    nc.vector.tensor_add(adt[:], adt[:], AD[:, :, 0])
                    e_t = pool.tile([P, sc], f32, tag="e")
                    nc.vector.tensor_tensor(out=e_t[:], in0=G[:, :, als_c],
                                            in1=adt[:], op=OP.add)
                    e_s = pool.tile([P, sc], f32, tag="es")
                    nc.vector.tensor_scalar(out=e_s[:], in0=e_t[:],
                                            scalar1=NEG, scalar2=None,
                                            op0=OP.mult)
                    nc.vector.tensor_max(e_t[:], e_t[:], e_s[:])
                    p_t = pool.tile([P, sc], f32, tag="p")
                    nc.scalar.activation(p_t[:], e_t[:], AF.Exp)

                    U = {}
                    for t in s["tiles"]:
                        U[t] = psum_u.tile([P, row], f32, tag="U", name="U")
                    done_once = set()
                    for ci, (t, _) in enumerate(s["chunks"]):
                        gc = base + ci
                        if "chunkmm" in ablate:
                            if t in done_once:
                                continue
                            done_once.add(t)
                            Mp0 = misc["mp_pool"].tile([P, P], f32, tag="Mp",
                                                       name="Mp0")
                            nc.vector.memset(Mp0[:, 0:1], 1.0)
                            nc.tensor.matmul(U[t][:], Mp0[:], G[:, ci, 0:row],
                                             start=True, stop=True)
                            continue
                        Mp = misc["mp_pool"].tile([P, P], g_dt, tag="Mp")
                        if "mpscale" in ablate:
                            nc.vector.memset(Mp[:, 0:1], 1.0)
                        elif MP_ALT and ci % 2 == 1:
                            nc.scalar.activation(
                                Mp[:], M[:, ci, :], AF.Copy,
                                scale=p_t[:, ci:ci + 1])
                        else:
                            nc.vector.tensor_scalar(
                                out=Mp[:], in0=M[:, ci, :],
                                scalar1=p_t[:, ci:ci + 1], scalar2=None,
                                op0=OP.mult)
                        if "f32r" in ablate or F32R:
                            nc.tensor.matmul(
                                U[t][:],
                                Mp[:].bitcast(mybir.dt.float32r),
                                G[:, ci, 0:row].bitcast(mybir.dt.float32r),
                                start=meta["chunk_start"][gc],
                                stop=meta["chunk_stop"][gc])
                        else:
                            nc.tensor.matmul(U[t][:], Mp[:], G[:, ci, 0:row],
                                             start=meta["chunk_start"][gc],
                                             stop=meta["chunk_stop"][gc])

                    for t in s["tiles"]:
                        s_inv = pool.tile([P, 1], f32, tag="sinv")
                        nc.vector.reciprocal(s_inv[:],
                                             U[t][:, ones_c:ones_c + 1])
                        if layer == 1:
                            relu1 = pool.tile([P, D_HID], f32, tag="relu1")
                            if with_bias1:
                                o1 = pool.tile([P, D_HID], f32, tag="o1")
                                nc.vector.tensor_scalar(
                                    out=o1[:], in0=U[t][:, 0:D_HID],
                                    scalar1=s_inv[:, 0:1], scalar2=None,
                                    op0=OP.mult)
                                nc.vector.tensor_add(o1[:], o1[:], b1_t[:])
                                nc.scalar.activation(relu1[:], o1[:], AF.Relu)
                            else:
                                nc.scalar.activation(
                                    relu1[:], U[t][:, 0:D_HID], AF.Relu,
                                    scale=s_inv[:, 0:1])
                            rT = misc["ps_t"].tile([P, P], f32, tag="rT")
                            nc.tensor.transpose(out=rT[:], in_=relu1[:],
                                                identity=ident_t[:])
                            rT_sb = pool.tile([P, P], f32, tag="rTsb")
                            nc.vector.tensor_copy(rT_sb[:], rT[:])
                            h2 = misc["ps_h"].tile([P, 18], f32, tag="h2")
                            nc.tensor.matmul(h2[:], rT_sb[:], w2_t[:],
                                             start=True, stop=True)
                            stg2 = pool.tile([P, ROW2], f32, tag="stg2")
                            nc.vector.tensor_copy(stg2[:, 0:18], h2[:])
                            nc.vector.memset(stg2[:, 18:19], 1.0)
                            side_writes.append(nc.sync.dma_start(
                                out=ag_in.ap()[t * P:(t + 1) * P, :],
                                in_=stg2[:]))
                        else:
                            o2 = pool.tile([P, D_OUT], f32, tag="o2")
                            nc.vector.tensor_scalar(
                                out=o2[:], in0=U[t][:, 0:D_OUT],
                                scalar1=s_inv[:, 0:1], scalar2=None,
                                op0=OP.mult)
                            if with_bias2:
                                nc.vector.tensor_add(o2[:], o2[:], b2_t[:])
                            B = pool.tile([P, N_GRAPHS], f32, tag="B")
                            nc.vector.tensor_tensor(
                                out=B[:],
                                in0=boff_t[:, t:t + 1]
                                    .broadcast_to([P, N_GRAPHS]),
                                in1=iota_t[:, 0:N_GRAPHS], op=OP.is_equal)
                            nc.tensor.matmul(misc["pool_ps"][:], B[:], o2[:],
                                             start=(t == 0),
                                             stop=(t == TILES_PER_CORE - 1))
                return gathers, ad_gathers, side_writes

            # ---------------- phase 2: layer 1 ----------------------------
            with (
                tc.tile_pool(name="l1", bufs=G_BUFS) as pool,
                tc.tile_pool(name="l1b", bufs=2) as pool2,
                tc.tile_pool(name="l1mp", bufs=MP_BUFS) as mp_pool,
                tc.tile_pool(name="l1u", bufs=U1_BUFS, space="PSUM") as psum_u,
                tc.tile_pool(name="l1t", bufs=2, space="PSUM") as ps_t,
                tc.tile_pool(name="l1h", bufs=8 - 2 - U1_BUFS,
                             space="PSUM") as ps_h,
            ):
                for _r in range(rep["l1"]):
                    gathers1, adg1, ag_writes = aggregation_layer(
                        1, pool, psum_u,
                        dict(mp_pool=mp_pool, ps_t=ps_t, ps_h=ps_h,
                             pool2=pool2))
                    for g in gathers1:
                        dep(g, fence0)
                    for g in adg1:
                        dep(g, fence1)

            # ---------------- phase 3: AllGather + expand ------------------
            if "cc" in ablate:
                cc1 = nc.sync.dma_start(out=ag_out.ap()[0:NODES_PER_CORE, :],
                                        in_=ag_in.ap())
            else:
                cc1 = nc.gpsimd.collective_compute(
                    "AllGather", OP.bypass, replica_groups=[core_ids],
                    ins=[ag_in[:]], outs=[ag_out[:]])
            dep(cc1, *ag_writes)
            ex1 = nc.sync.dma_start(out=t2lo.ap()[:, 0:ROW2],
                                    in_=ag_out.ap()[0:NLO, :])
            ex2 = nc.sync.dma_start(out=t2hi.ap()[:, 0:ROW2],
                                    in_=ag_out.ap()[NLO:NPAD, :])
            with nc.allow_non_contiguous_dma(
                    reason="4B/row ald column expand"):
                ex3 = nc.sync.dma_start(out=t2ad.ap()[:, 0:1],
                                        in_=ag_out.ap()[0:NAD, 17:18])
                ex4 = nc.sync.dma_start(out=t2ad.ap()[:, 1:2],
                                        in_=ag_out.ap()[NAD:NPAD, 17:18])
            dep(ex1, cc1)
            dep(ex2, cc1)
            dep(ex3, cc1)
            dep(ex4, cc1)
            fence2 = nc.sync.nop(nofuse=True, hint="fence_p3")
            dep(fence2, ex1, ex2, ex3, ex4)

            # ---------------- phase 4: layer 2 + pooling -------------------
            with (
                tc.tile_pool(name="l2", bufs=G_BUFS) as pool,
                tc.tile_pool(name="l2b", bufs=2) as pool2,
                tc.tile_pool(name="l2mp", bufs=MP_BUFS) as mp_pool,
                tc.tile_pool(name="l2u", bufs=U2_BUFS, space="PSUM") as psum_u,
                tc.tile_pool(name="poolps", bufs=1, space="PSUM") as pps,
            ):
                pool_ps = pps.tile([N_GRAPHS, D_OUT], f32)
                for _r in range(rep["l2"]):
                    gathers2, adg2, _ = aggregation_layer(
                        2, pool, psum_u,
                        dict(mp_pool=mp_pool, pool_ps=pool_ps, pool2=pool2))
                    for g in gathers2 + adg2:
                        dep(g, fence2)
                    if _r == 0 and rep["l2"] > 1:
                        pool_ps2 = pps.tile([N_GRAPHS, D_OUT], f32,
                                            name="poolps2")
                        pool_ps = pool_ps2

                # -------------- phase 5: reduce + log_softmax --------------
                pp_sb = pool.tile([N_GRAPHS, D_OUT], f32)
                nc.vector.tensor_copy(pp_sb[:], pool_ps[:])
                w_ar = nc.sync.dma_start(out=ar_in.ap(), in_=pp_sb[:])
                if "cc" in ablate:
                    cc2 = nc.sync.dma_start(out=ar_out.ap(), in_=ar_in.ap())
                else:
                    cc2 = nc.gpsimd.collective_compute(
                        "AllReduce", OP.add, replica_groups=[core_ids],
                        ins=[ar_in[:]], outs=[ar_out[:]])
                dep(cc2, w_ar)
                red = pool.tile([N_GRAPHS, D_OUT], f32)
                r_ld = nc.sync.dma_start(out=red[:], in_=ar_out.ap())
                dep(r_ld, cc2)
                mean = pool.tile([N_GRAPHS, D_OUT], f32)
                nc.vector.tensor_scalar(out=mean[:], in0=red[:],
                                        scalar1=rcnt_t[:, 0:1], scalar2=None,
                                        op0=OP.mult)
                mx = pool.tile([N_GRAPHS, 1], f32)
                nc.vector.tensor_reduce(mx[:], mean[:],
                                        axis=mybir.AxisListType.X, op=OP.max)
                xm = pool.tile([N_GRAPHS, D_OUT], f32)
                nc.vector.tensor_scalar(out=xm[:], in0=mean[:],
                                        scalar1=mx[:, 0:1], scalar2=None,
                                        op0=OP.subtract)
                ex = pool.tile([N_GRAPHS, D_OUT], f32)
                nc.scalar.activation(ex[:], xm[:], AF.Exp)
                ssum = pool.tile([N_GRAPHS, 1], f32)
                nc.vector.tensor_reduce(ssum[:], ex[:],
                                        axis=mybir.AxisListType.X, op=OP.add)
                lse = pool.tile([N_GRAPHS, 1], f32)
                nc.scalar.activation(lse[:], ssum[:], AF.Ln)
                fin = pool.tile([N_GRAPHS, D_OUT], f32)
                nc.vector.tensor_scalar(out=fin[:], in0=xm[:],
                                        scalar1=lse[:, 0:1], scalar2=None,
                                        op0=OP.subtract)
                nc.sync.dma_start(out=out_d.ap(), in_=fin[:])

    nc.compile()
    return nc


# --------------------------------------------------------------- entry point
_CACHE = {}


def prepare(inputs):
    """Host preprocessing + (cached) program build. Returns (nc, in_maps)."""
    x = np.asarray(inputs["x"], np.float32)
    edge_index = np.asarray(inputs["edge_index"])
    batch = np.asarray(inputs["batch"])
    W1 = np.asarray(inputs["W1"], np.float32)
    a1s = np.asarray(inputs["a1_src"], np.float32)
    a1d = np.asarray(inputs["a1_dst"], np.float32)
    b1 = np.asarray(inputs["b1"], np.float32)
    W2 = np.asarray(inputs["W2"], np.float32)
    a2s = np.asarray(inputs["a2_src"], np.float32)
    a2d = np.asarray(inputs["a2_dst"], np.float32)
    b2 = np.asarray(inputs["b2"], np.float32)

    meta, per_core = preprocess(edge_index, batch)
    with_b1 = bool(np.abs(b1).max() > 0)
    with_b2 = bool(np.abs(b2).max() > 0)

    key = (meta["TC"], with_b1, with_b2, meta["idx_lo_cols"],
           meta["idx_hi_cols"], tuple(int(v) for v in meta["CLO"]),
           tuple(int(v) for v in meta["CHI"]))
    if key not in _CACHE:
        _CACHE[key] = build_program(meta, with_b1, with_b2)
    nc = _CACHE[key]

    xT = np.zeros((P, NPAD), np.float32)
    xT[:, :N_NODES] = x.T
    w1aug = np.concatenate([W1, (W1 @ a1s)[:, None], (W1 @ a1d)[:, None]],
                           axis=1).astype(np.float32)
    w2aug = np.concatenate([W2, (W2 @ a2s)[:, None], (W2 @ a2d)[:, None]],
                           axis=1).astype(np.float32)
    iota = np.tile(np.arange(P, dtype=np.int16), (P, 1))
    cnt = np.bincount(batch.astype(np.int64), minlength=N_GRAPHS)
    rcnt = (1.0 / np.maximum(cnt, 1)).astype(np.float32)[:, None]

    in_maps = []
    for k in range(NCORES):
        m = dict(xT=xT, w1aug=w1aug, w2aug=w2aug,
                 idx_lo=per_core[k]["idx_lo"], idx_hi=per_core[k]["idx_hi"],
                 idx_ad=per_core[k]["idx_ad"],
                 admask=per_core[k]["admask"],
                 dstoff=per_core[k]["dstoff"],
                 batchoff=per_core[k]["batchoff"],
                 iota=iota, recip_cnt=rcnt)
        if with_b1:
            m["b1b"] = np.tile(b1[None, :], (P, 1)).astype(np.float32)
        if with_b2:
            m["b2b"] = np.tile(b2[None, :], (P, 1)).astype(np.float32)
        in_maps.append(m)
    return nc, in_maps


def kernel(**inputs) -> np.ndarray:
    nc, in_maps = prepare(inputs)
    res = run_bass_kernel_spmd(nc, in_maps, list(range(NCORES)))
    return np.asarray(res.results[0]["out"], np.float32)



# revision 2
# speedup vs baseline: 3.4435x; 3.4435x over previous
"""Trainium2 Bass kernel for 2-layer GAT (nn_GAT_5970004541990).

Sharding: nodes padded 50000 -> 50176, sharded by destination across 8 cores
(49 tiles of 128). Self-loops added. Host does index preprocessing only.

Design (1.59ms on HW vs 4.38ms for the previous table-based version):
- Aggregation in x-space: T1 rows = [x_bf16(128) | als1 | ones] (512B, host
  prefills x cols); layer-1 output h1 = (U_x/s) @ W1 computed per dst tile.
- bf16 tables + bf16 chunk matmuls (PE 4x) + bf16 M/Mp.
- NO per-edge ald gathers: ald1 via one 128-row gather from ald_tab[392,128]
  + PE transpose -> ald_sb[P,49]; ald2 computed locally per dst tile.
- Layer-2 table IS the AllGather output (ag rows 256B = [h2(16)|als2|ones|pad])
  gathered directly; no expand step.
- dma_gather descriptor-gen is the bottleneck (~9.5ns/row serialized);
  4 SWDGE queues + round-robin gives ~2x overlap.
"""
import sys

sys.path.insert(0, "/opt/trn_rl_repo")

import numpy as np
import ml_dtypes

import concourse.bass as bass
import concourse.mybir as mybir
import concourse.tile as tile
from concourse.tile_rust import add_dep_helper
from concourse import bacc
from concourse.bass_utils import run_bass_kernel_spmd
from concourse.masks import make_identity

# ---------------------------------------------------------------- constants
N_NODES = 50000
N_GRAPHS = 64
D_HID = 128
D_OUT = 16
NEG = 0.2

P = 128
NCORES = 8
NPAD = 50176                     # 8 * 49 * 128
NODES_PER_CORE = NPAD // NCORES  # 6272
TILES_PER_CORE = NODES_PER_CORE // P  # 49
GLOBAL_TILES = NPAD // P         # 392
NLO = 32768                      # int16 gather index limit
NHI = NPAD - NLO

S_TILES = 2                      # dst tiles per super-tile
G_BUFS = 5
MP_BUFS = 12
U1_BUFS = 4
U2_BUFS = 4
XB_P1 = 8                        # phase-1 tiles per batch
NQ = 4                           # SWDGE queues
MAXCH = 12                       # max chunks per gather piece (ring pressure)
DMA_SCRATCH = 32768              # SWDGE descriptor carveout bytes/partition

ELEM1 = 256                      # T1 row: 256 bf16 = 512B
ELEM2 = 128                      # ag row: 128 bf16 = 256B
ROW1 = 130                       # T1 meaningful: x 0..127, als 128, ones 129
ROW2 = 18                        # ag meaningful: h2 0..15, als2 16, ones 17

f32 = mybir.dt.float32
i16 = mybir.dt.int16
bf16 = mybir.dt.bfloat16


def _ceil(a, b):
    return -(-a // b)


def _wrap_idx(flat):
    v16 = flat.astype(np.uint16).view(np.int16)
    return np.tile(v16.reshape(-1, 16).T, (8, 1))


def _pad_to(a, n, fill):
    out = np.full(n, fill, a.dtype)
    out[:len(a)] = a
    return out


# ------------------------------------------------------------ host indexing
def preprocess(edge_index, batch):
    src = np.concatenate(
        [edge_index[0].astype(np.int64), np.arange(NPAD, dtype=np.int64)])
    dst = np.concatenate(
        [edge_index[1].astype(np.int64), np.arange(NPAD, dtype=np.int64)])
    core = dst // NODES_PER_CORE

    cores = []
    nlo = np.zeros((NCORES, TILES_PER_CORE), np.int64)
    nhi = np.zeros((NCORES, TILES_PER_CORE), np.int64)
    for k in range(NCORES):
        m = core == k
        s_k = src[m]
        d_k = dst[m] - k * NODES_PER_CORE
        t_k = d_k // P
        o_k = (d_k % P).astype(np.int16)
        h_k = s_k >= NLO
        lo, hi = {}, {}
        for t in range(TILES_PER_CORE):
            ml = (t_k == t) & ~h_k
            mh = (t_k == t) & h_k
            lo[t] = (s_k[ml], o_k[ml])
            hi[t] = (s_k[mh] - NLO, o_k[mh])
            nlo[k, t] = ml.sum()
            nhi[k, t] = mh.sum()
        cores.append((lo, hi))

    CLO = np.maximum(_ceil(nlo, P).max(axis=0), 1)
    CHI = _ceil(nhi, P).max(axis=0)

    supers = []
    chunk_tiles, chunk_start, chunk_stop = [], [], []
    lo_c = hi_c = 0
    for s0 in range(0, TILES_PER_CORE, S_TILES):
        tiles = list(range(s0, min(s0 + S_TILES, TILES_PER_CORE)))
        chunks = [(t, 0) for t in tiles for _ in range(int(CLO[t]))]
        n_lo = len(chunks)
        chunks += [(t, 1) for t in tiles for _ in range(int(CHI[t]))]
        n_hi = len(chunks) - n_lo
        first, last = {}, {}
        for ci, (t, _) in enumerate(chunks):
            first.setdefault(t, ci)
            last[t] = ci
        base = len(chunk_tiles)
        supers.append(dict(tiles=tiles, chunks=chunks, n_lo=n_lo, n_hi=n_hi,
                           base=base, lo_col0=lo_c, hi_col0=hi_c))
        lo_c += n_lo * 8
        hi_c += n_hi * 8
        for ci, (t, _) in enumerate(chunks):
            chunk_tiles.append(t)
            chunk_start.append(ci == first[t])
            chunk_stop.append(ci == last[t])
    TC = len(chunk_tiles)

    batch_pad = np.full(NPAD, 127, np.int64)
    batch_pad[:N_NODES] = batch.astype(np.int64)
    per_core = []
    for k in range(NCORES):
        lo, hi = cores[k]
        dstoff = np.full((TC, P), -1, np.int16)
        lo_cols, hi_cols = [], []
        for s in supers:
            lo_flat, hi_flat = [], []
            used = {}
            for t in s["tiles"]:
                lo_flat.append(_pad_to(lo[t][0], int(CLO[t]) * P, 0))
                if CHI[t]:
                    hi_flat.append(_pad_to(hi[t][0], int(CHI[t]) * P, 0))
            for ci, (t, h) in enumerate(s["chunks"]):
                row = used.get((t, h), 0)
                used[(t, h)] = row + 1
                offs = (lo if h == 0 else hi)[t][1]
                want = (int(CLO[t]) if h == 0 else int(CHI[t])) * P
                o = _pad_to(offs, want, np.int16(-1))
                dstoff[s["base"] + ci] = o[row * P:(row + 1) * P]
            lo_cols.append(_wrap_idx(np.concatenate(lo_flat)))
            if hi_flat:
                hi_cols.append(_wrap_idx(np.concatenate(hi_flat)))
        idx_lo = np.concatenate(lo_cols, axis=1)
        idx_hi = (np.concatenate(hi_cols, axis=1) if hi_cols
                  else np.zeros((P, 1), np.int16))
        tiles_g = _pad_to(
            np.arange(k * TILES_PER_CORE, (k + 1) * TILES_PER_CORE,
                      dtype=np.int64), P, 0)
        boff = batch_pad[k * NODES_PER_CORE:(k + 1) * NODES_PER_CORE]
        per_core.append(dict(
            idx_lo=np.ascontiguousarray(idx_lo),
            idx_hi=np.ascontiguousarray(idx_hi),
            tileidx=np.ascontiguousarray(_wrap_idx(tiles_g)),
            dstoff=np.ascontiguousarray(dstoff.T),
            batchoff=np.ascontiguousarray(
                boff.reshape(TILES_PER_CORE, P).T.astype(np.int16))))

    meta = dict(supers=supers, CLO=CLO, CHI=CHI, TC=TC,
                chunk_start=chunk_start, chunk_stop=chunk_stop,
                idx_lo_cols=per_core[0]["idx_lo"].shape[1],
                idx_hi_cols=per_core[0]["idx_hi"].shape[1])
    return meta, per_core


# ------------------------------------------------------------- bass program
def build_program(meta, with_b1, with_b2):
    nc = bacc.Bacc("TRN2", target_bir_lowering=False, debug=False,
                   num_devices=NCORES, num_swdge_queues=NQ,
                   dynamic_dma_scratch_size=DMA_SCRATCH)
    TC = meta["TC"]
    supers = meta["supers"]
    AF = mybir.ActivationFunctionType
    OP = mybir.AluOpType
    core_ids = list(range(NCORES))

    # T1 host-prefilled x cols; kernel writes cols 128:130 (idempotent).
    t1_d = nc.dram_tensor("t1", [NPAD, ELEM1], bf16, kind="ExternalInput")
    xT_d = nc.dram_tensor("xTb", [P, NPAD], bf16, kind="ExternalInput")
    wad_d = nc.dram_tensor("wad", [P, 2], bf16, kind="ExternalInput")
    w1_d = nc.dram_tensor("w1b", [P, D_HID], bf16, kind="ExternalInput")
    w2_d = nc.dram_tensor("w2aug", [P, ROW2], bf16, kind="ExternalInput")
    ilo_d = nc.dram_tensor("idx_lo", [P, meta["idx_lo_cols"]], i16,
                           kind="ExternalInput")
    ihi_d = nc.dram_tensor("idx_hi", [P, meta["idx_hi_cols"]], i16,
                           kind="ExternalInput")
    doff_d = nc.dram_tensor("dstoff", [P, TC], i16, kind="ExternalInput")
    tidx_d = nc.dram_tensor("tileidx", [P, 8], i16, kind="ExternalInput")
    boff_d = nc.dram_tensor("batchoff", [P, TILES_PER_CORE], i16,
                            kind="ExternalInput")
    iota_d = nc.dram_tensor("iota", [P, P], i16, kind="ExternalInput")
    rcnt_d = nc.dram_tensor("recip_cnt", [N_GRAPHS, 1], f32,
                            kind="ExternalInput")
    if with_b1:
        b1_d = nc.dram_tensor("b1col", [P, 1], f32, kind="ExternalInput")
    if with_b2:
        b2_d = nc.dram_tensor("b2b", [P, D_OUT], f32, kind="ExternalInput")
    out_d = nc.dram_tensor("out", [N_GRAPHS, D_OUT], f32,
                           kind="ExternalOutput")

    import os
    dbg = bool(int(os.environ.get("K2_DEBUG", "0")))
    if dbg:
        dbg_als = nc.dram_tensor("dbg_als", [NPAD, 2], bf16,
                                 kind="ExternalOutput")
        dbg_ald = nc.dram_tensor("dbg_ald", [GLOBAL_TILES, P], f32,
                                 kind="ExternalOutput")
        dbg_ag = nc.dram_tensor("dbg_ag", [NODES_PER_CORE, ELEM2], bf16,
                                kind="ExternalOutput")
    ald_tab = nc.dram_tensor("ald_tab", [GLOBAL_TILES, P], f32)
    ag_in = nc.dram_tensor("ag_in", [NODES_PER_CORE, ELEM2], bf16)
    ag_out = nc.dram_tensor("ag_out", [NPAD, ELEM2], bf16,
                            addr_space="Shared")
    ar_in = nc.dram_tensor("ar_in", [N_GRAPHS, D_OUT], f32)
    ar_out = nc.dram_tensor("ar_out", [N_GRAPHS, D_OUT], f32,
                            addr_space="Shared")

    def dep(after, *before):
        for b in before:
            add_dep_helper(after.ins, b.ins, reason="phase order")

    qload = [0] * NQ

    def pick_q(rows):
        q = min(range(NQ), key=lambda i: qload[i])
        qload[q] += rows
        return q

    with tile.TileContext(nc) as tc:
        with tc.tile_pool(name="res", bufs=1) as res:
            iota_t = res.tile([P, P], i16)
            nc.sync.dma_start(out=iota_t[:], in_=iota_d.ap())
            doff_t = res.tile([P, TC], i16)
            nc.sync.dma_start(out=doff_t[:], in_=doff_d.ap())
            ilo_t = res.tile([P, meta["idx_lo_cols"]], i16)
            nc.sync.dma_start(out=ilo_t[:], in_=ilo_d.ap())
            ihi_t = res.tile([P, meta["idx_hi_cols"]], i16)
            nc.sync.dma_start(out=ihi_t[:], in_=ihi_d.ap())
            tidx_t = res.tile([P, 8], i16)
            nc.sync.dma_start(out=tidx_t[:], in_=tidx_d.ap())
            boff_t = res.tile([P, TILES_PER_CORE], i16)
            nc.sync.dma_start(out=boff_t[:], in_=boff_d.ap())
            wad_t = res.tile([P, 2], bf16)
            nc.sync.dma_start(out=wad_t[:], in_=wad_d.ap())
            w1_t = res.tile([P, D_HID], bf16)
            nc.sync.dma_start(out=w1_t[:], in_=w1_d.ap())
            w2_t = res.tile([P, ROW2], bf16)
            nc.sync.dma_start(out=w2_t[:], in_=w2_d.ap())
            rcnt_t = res.tile([N_GRAPHS, 1], f32)
            nc.sync.dma_start(out=rcnt_t[:], in_=rcnt_d.ap())
            ident_t = res.tile([P, P], f32)
            make_identity(nc, ident_t[:])
            ident_b = res.tile([P, P], bf16)
            make_identity(nc, ident_b[:])
            if with_b1:
                b1_t = res.tile([P, 1], f32)
                nc.sync.dma_start(out=b1_t[:], in_=b1_d.ap())
            if with_b2:
                b2_t = res.tile([P, D_OUT], f32)
                nc.sync.dma_start(out=b2_t[:], in_=b2_d.ap())
            ald_sb = res.tile([P, TILES_PER_CORE], f32, name="ald_sb")
            ald2_sb = res.tile([P, TILES_PER_CORE], f32, name="ald2_sb")
            # BC_t[p, t, j] = ald[tile t][node j] (replicated over partitions)
            BC_t = res.tile([P, TILES_PER_CORE, P], bf16, name="BC_t")
            ones_row = res.tile([1, P], bf16, name="ones_row")
            nc.vector.memset(ones_row[:], 1.0)

            # ---------------- phase 1: als/ald columns --------------------
            tab_writes = []
            XB = XB_P1
            with (
                tc.tile_pool(name="p1", bufs=3) as p1,
                tc.tile_pool(name="p1ps", bufs=4, space="PSUM") as p1ps,
                tc.tile_pool(name="p1tp", bufs=2, space="PSUM") as p1tp,
            ):
                for gb in range(0, GLOBAL_TILES, XB):
                    nb = min(XB, GLOBAL_TILES - gb)
                    xt = p1.tile([P, XB * P], bf16, tag="xt")
                    nc.sync.dma_start(
                        out=xt[:, 0:nb * P],
                        in_=xT_d.ap()[:, gb * P:(gb + nb) * P])
                    st_ad = p1.tile([P, XB, 2], bf16, tag="st_ad")
                    aldstg = p1.tile([P, XB], f32, tag="aldstg")
                    for j in range(nb):
                        ad_ps = p1ps.tile([P, 2], f32, tag="ad")
                        nc.tensor.matmul(ad_ps[:], xt[:, j * P:(j + 1) * P],
                                         wad_t[:], start=True, stop=True)
                        nc.scalar.activation(st_ad[:, j, 0:1], ad_ps[:, 0:1],
                                             AF.Copy)
                        nc.vector.tensor_copy(aldstg[:, j:j + 1],
                                              ad_ps[:, 1:2])
                    nc.vector.memset(st_ad[:, 0:nb, 1:2], 1.0)
                    with nc.allow_non_contiguous_dma(
                            reason="4B/row als+ones column write"):
                        w1w = nc.sync.dma_start(
                            out=t1_d.ap()[gb * P:(gb + nb) * P, 128:130]
                                .rearrange("(j p) c -> p j c", p=P),
                            in_=st_ad[:, 0:nb, :])
                    tr_ps = p1tp.tile([XB, P], f32, tag="tr")
                    nc.tensor.transpose(out=tr_ps[0:nb, :],
                                        in_=aldstg[:, 0:nb],
                                        identity=ident_t[:])
                    trsb = p1.tile([XB, P], f32, tag="trsb")
                    nc.vector.tensor_copy(trsb[0:nb, :], tr_ps[0:nb, :])
                    w2w = nc.sync.dma_start(
                        out=ald_tab.ap()[gb:gb + nb, :],
                        in_=trsb[0:nb, :])
                    tab_writes.append(w1w)
                    tab_writes.append(w2w)

            fence0 = nc.sync.nop(nofuse=True, hint="fence_p1")
            dep(fence0, *tab_writes)
            if dbg:
                with nc.allow_non_contiguous_dma(reason="debug dump"):
                    d1 = nc.sync.dma_start(out=dbg_als.ap(),
                                           in_=t1_d.ap()[:, 128:130])
                d2 = nc.sync.dma_start(out=dbg_ald.ap(), in_=ald_tab.ap())
                dep(d1, fence0)
                dep(d2, fence0)

            # ---------------- ald1 -> ald_sb ------------------------------
            with tc.tile_pool(name="alds", bufs=1) as apool, \
                 tc.tile_pool(name="aldps", bufs=1, space="PSUM") as aps:
                ald_g = apool.tile([P, 1, P], f32)
                g_ald = nc.gpsimd.dma_gather(
                    out_ap=ald_g[:], in_ap=ald_tab.ap(),
                    idxs_ap=tidx_t[:, 0:8], num_idxs=P, num_idxs_reg=P,
                    elem_size=P, single_packet=False, queue_num=0)
                dep(g_ald, fence0)
                aldT_ps = aps.tile([P, P], f32)
                nc.tensor.transpose(out=aldT_ps[:], in_=ald_g[:, 0, :],
                                    identity=ident_t[:])
                nc.vector.tensor_copy(ald_sb[:],
                                      aldT_ps[:, 0:TILES_PER_CORE])

            # --------------- BC build: outer-product per tile -------------
            def build_bc(alds, tag):
                with (
                    tc.tile_pool(name=f"bcps{tag}", bufs=2,
                                 space="PSUM") as bcps,
                    tc.tile_pool(name=f"bcsb{tag}", bufs=2) as bsb,
                ):
                    for t in range(TILES_PER_CORE):
                        rowp = bcps.tile([1, P], f32, tag="rowp")
                        nc.tensor.transpose(out=rowp[:],
                                            in_=alds[:, t:t + 1],
                                            identity=ident_t[:])
                        rowsb = bsb.tile([1, P], bf16, tag="rowsb")
                        nc.vector.tensor_copy(rowsb[:], rowp[:])
                        bcp = bcps.tile([P, P], f32, tag="bcp")
                        nc.tensor.matmul(bcp[:], ones_row[:], rowsb[:],
                                         start=True, stop=True)
                        nc.vector.tensor_copy(BC_t[:, t, :], bcp[:])

            # --------------- shared aggregation loop ----------------------
            def aggregation_layer(layer, pool, pool2, mp_pool, psum_u, misc):
                if layer == 1:
                    elem, mm_hi, als_c, ones_c = ELEM1, ROW1, 128, 129
                    tab = t1_d
                    alds = ald_sb
                else:
                    elem, mm_hi, als_c, ones_c = ELEM2, ROW2, 16, 17
                    tab = ag_out
                    alds = ald2_sb
                gathers = []
                side_writes = []

                for s in supers:
                    sc = len(s["chunks"])
                    n_lo, n_hi, base = s["n_lo"], s["n_hi"], s["base"]
                    G = pool.tile([P, sc, elem], bf16, tag="G")
                    for p0 in range(0, n_lo, MAXCH):
                        p1 = min(p0 + MAXCH, n_lo)
                        c0 = s["lo_col0"] + p0 * 8
                        g1 = nc.gpsimd.dma_gather(
                            out_ap=G[:, p0:p1, :], in_ap=tab.ap()[0:NLO, :],
                            idxs_ap=ilo_t[:, c0:c0 + (p1 - p0) * 8],
                            num_idxs=(p1 - p0) * P,
                            num_idxs_reg=(p1 - p0) * P,
                            elem_size=elem, single_packet=False,
                            queue_num=pick_q(p1 - p0))
                        gathers.append(g1)
                    for p0 in range(0, n_hi, MAXCH):
                        p1 = min(p0 + MAXCH, n_hi)
                        c0 = s["hi_col0"] + p0 * 8
                        g2 = nc.gpsimd.dma_gather(
                            out_ap=G[:, n_lo + p0:n_lo + p1, :],
                            in_ap=tab.ap()[NLO:NPAD, :],
                            idxs_ap=ihi_t[:, c0:c0 + (p1 - p0) * 8],
                            num_idxs=(p1 - p0) * P,
                            num_idxs_reg=(p1 - p0) * P,
                            elem_size=elem, single_packet=False,
                            queue_num=pick_q(p1 - p0))
                        gathers.append(g2)

                    M = pool2.tile([P, sc, P], bf16, tag="M")
                    nc.vector.tensor_tensor(
                        out=M[:],
                        in0=doff_t[:, base:base + sc].unsqueeze(2)
                            .broadcast_to([P, sc, P]),
                        in1=iota_t[:].unsqueeze(1).broadcast_to([P, sc, P]),
                        op=OP.is_equal)

                    # per contiguous same-tile run: e = als + ald ; lrelu ; exp
                    p_t = pool.tile([P, sc], f32, tag="p")
                    runs = []
                    for ci, (t, _) in enumerate(s["chunks"]):
                        if runs and runs[-1][0] == t and runs[-1][2] == ci - 1:
                            runs[-1][2] = ci
                        else:
                            runs.append([t, ci, ci])
                    for t, c0, c1 in runs:
                        nch = c1 - c0 + 1
                        tmp = pool2.tile([P, nch, P], bf16, tag="tmp")
                        nc.vector.tensor_tensor(
                            out=tmp[:], in0=M[:, c0:c1 + 1, :],
                            in1=BC_t[:, t, :].unsqueeze(1)
                                .broadcast_to([P, nch, P]),
                            op=OP.mult)
                        alde = pool.tile([P, nch, 1], f32, tag="alde",
                                         name="alde")
                        nc.vector.tensor_reduce(alde[:], tmp[:],
                                                axis=mybir.AxisListType.X,
                                                op=OP.add)
                        e_t = pool.tile([P, nch], f32, tag="e", name="e")
                        nc.vector.tensor_tensor(
                            out=e_t[:], in0=G[:, c0:c1 + 1, als_c],
                            in1=alde[:, :, 0], op=OP.add)
                        e_s = pool.tile([P, nch], f32, tag="es", name="es")
                        nc.vector.tensor_scalar(out=e_s[:], in0=e_t[:],
                                                scalar1=NEG, scalar2=None,
                                                op0=OP.mult)
                        nc.vector.tensor_max(e_t[:], e_t[:], e_s[:])
                        nc.scalar.activation(p_t[:, c0:c1 + 1], e_t[:],
                                             AF.Exp)

                    U = {}
                    for t in s["tiles"]:
                        U[t] = psum_u.tile([P, mm_hi], f32, tag="U", name="U")
                    for ci, (t, _) in enumerate(s["chunks"]):
                        gc = base + ci
                        Mp = mp_pool.tile([P, P], bf16, tag="Mp")
                        if ci % 3 == 0:
                            nc.vector.tensor_scalar(
                                out=Mp[:], in0=M[:, ci, :],
                                scalar1=p_t[:, ci:ci + 1], scalar2=None,
                                op0=OP.mult)
                        else:
                            nc.scalar.activation(
                                Mp[:], M[:, ci, :], AF.Copy,
                                scale=p_t[:, ci:ci + 1])
                        nc.tensor.matmul(U[t][:], Mp[:], G[:, ci, 0:mm_hi],
                                         start=meta["chunk_start"][gc],
                                         stop=meta["chunk_stop"][gc])

                    for t in s["tiles"]:
                        s_inv = pool.tile([P, 1], f32, tag="sinv")
                        nc.vector.reciprocal(s_inv[:],
                                             U[t][:, ones_c:ones_c + 1])
                        if layer == 1:
                            # Uxn = (U_x * s_inv) bf16 ; transpose ;
                            # h1T = W1^T @ UxT ; relu ; h2aug = relu1 @ W2aug
                            uxn = pool.tile([P, P], bf16, tag="uxn")
                            nc.scalar.activation(uxn[:], U[t][:, 0:P],
                                                 AF.Copy,
                                                 scale=s_inv[:, 0:1])
                            uxT_ps = misc["ps_t"].tile([P, P], bf16,
                                                       tag="uxT")
                            nc.tensor.transpose(out=uxT_ps[:], in_=uxn[:],
                                                identity=ident_b[:])
                            uxT = pool.tile([P, P], bf16, tag="uxTs")
                            nc.vector.tensor_copy(uxT[:], uxT_ps[:])
                            h1T_ps = misc["ps_h"].tile([P, P], f32, tag="h1T")
                            nc.tensor.matmul(h1T_ps[:], w1_t[:], uxT[:],
                                             start=True, stop=True)
                            relu1T = pool.tile([P, P], bf16, tag="r1T")
                            if with_b1:
                                nc.scalar.activation(relu1T[:], h1T_ps[:],
                                                     AF.Relu,
                                                     bias=b1_t[:, 0:1])
                            else:
                                nc.scalar.activation(relu1T[:], h1T_ps[:],
                                                     AF.Relu)
                            h2_ps = misc["ps_h2"].tile([P, ROW2], f32,
                                                       tag="h2")
                            nc.tensor.matmul(h2_ps[:], relu1T[:], w2_t[:],
                                             start=True, stop=True)
                            nc.vector.tensor_copy(ald2_sb[:, t:t + 1],
                                                  h2_ps[:, 17:18])
                            stg2 = pool.tile([P, ELEM2], bf16, tag="stg2")
                            nc.vector.tensor_copy(stg2[:, 0:17],
                                                  h2_ps[:, 0:17])
                            nc.vector.memset(stg2[:, 17:18], 1.0)
                            side_writes.append(nc.sync.dma_start(
                                out=ag_in.ap()[t * P:(t + 1) * P, :],
                                in_=stg2[:]))
                        else:
                            o2 = pool.tile([P, D_OUT], bf16, tag="o2")
                            if with_b2:
                                o2f = pool.tile([P, D_OUT], f32, tag="o2f")
                                nc.vector.tensor_scalar(
                                    out=o2f[:], in0=U[t][:, 0:D_OUT],
                                    scalar1=s_inv[:, 0:1], scalar2=None,
                                    op0=OP.mult)
                                nc.vector.tensor_add(o2f[:], o2f[:], b2_t[:])
                                nc.vector.tensor_copy(o2[:], o2f[:])
                            else:
                                nc.scalar.activation(o2[:],
                                                     U[t][:, 0:D_OUT],
                                                     AF.Copy,
                                                     scale=s_inv[:, 0:1])
                            B = pool.tile([P, N_GRAPHS], bf16, tag="B")
                            nc.vector.tensor_tensor(
                                out=B[:],
                                in0=boff_t[:, t:t + 1]
                                    .broadcast_to([P, N_GRAPHS]),
                                in1=iota_t[:, 0:N_GRAPHS], op=OP.is_equal)
                            nc.tensor.matmul(misc["pool_ps"][:], B[:], o2[:],
                                             start=(t == 0),
                                             stop=(t == TILES_PER_CORE - 1))
                return gathers, side_writes

            build_bc(ald_sb, "1")

            # ---------------- layer 1 -------------------------------------
            with (
                tc.tile_pool(name="l1", bufs=G_BUFS) as pool,
                tc.tile_pool(name="l1b", bufs=3) as pool2,
                tc.tile_pool(name="l1mp", bufs=MP_BUFS) as mp_pool,
                tc.tile_pool(name="l1u", bufs=U1_BUFS, space="PSUM") as psum_u,
                tc.tile_pool(name="l1t", bufs=2, space="PSUM") as ps_t,
                tc.tile_pool(name="l1h", bufs=1, space="PSUM") as ps_h,
                tc.tile_pool(name="l1h2", bufs=1, space="PSUM") as ps_h2,
            ):
                gathers1, ag_writes = aggregation_layer(
                    1, pool, pool2, mp_pool, psum_u,
                    dict(ps_t=ps_t, ps_h=ps_h, ps_h2=ps_h2))
                for g in gathers1:
                    dep(g, fence0)

            # ---------------- AllGather -----------------------------------
            cc1 = nc.gpsimd.collective_compute(
                "AllGather", mybir.AluOpType.bypass,
                replica_groups=[core_ids],
                ins=[ag_in[:]], outs=[ag_out[:]])
            dep(cc1, *ag_writes)
            if dbg:
                d3 = nc.sync.dma_start(out=dbg_ag.ap(), in_=ag_in.ap())
                dep(d3, *ag_writes)

            build_bc(ald2_sb, "2")

            # ---------------- layer 2 + pooling ---------------------------
            with (
                tc.tile_pool(name="l2", bufs=G_BUFS) as pool,
                tc.tile_pool(name="l2b", bufs=3) as pool2,
                tc.tile_pool(name="l2mp", bufs=MP_BUFS) as mp_pool,
                tc.tile_pool(name="l2u", bufs=U2_BUFS, space="PSUM") as psum_u,
                tc.tile_pool(name="poolps", bufs=1, space="PSUM") as pps,
            ):
                pool_ps = pps.tile([N_GRAPHS, D_OUT], f32)
                gathers2, _ = aggregation_layer(
                    2, pool, pool2, mp_pool, psum_u, dict(pool_ps=pool_ps))
                for g in gathers2:
                    dep(g, cc1)

                # -------------- reduce + log_softmax ----------------------
                pp_sb = pool.tile([N_GRAPHS, D_OUT], f32)
                nc.vector.tensor_copy(pp_sb[:], pool_ps[:])
                w_ar = nc.sync.dma_start(out=ar_in.ap(), in_=pp_sb[:])
                cc2 = nc.gpsimd.collective_compute(
                    "AllReduce", mybir.AluOpType.add,
                    replica_groups=[core_ids],
                    ins=[ar_in[:]], outs=[ar_out[:]])
                dep(cc2, w_ar)
                red = pool.tile([N_GRAPHS, D_OUT], f32)
                r_ld = nc.sync.dma_start(out=red[:], in_=ar_out.ap())
                dep(r_ld, cc2)
                mean = pool.tile([N_GRAPHS, D_OUT], f32)
                nc.vector.tensor_scalar(out=mean[:], in0=red[:],
                                        scalar1=rcnt_t[:, 0:1], scalar2=None,
                                        op0=OP.mult)
                mx = pool.tile([N_GRAPHS, 1], f32)
                nc.vector.tensor_reduce(mx[:], mean[:],
                                        axis=mybir.AxisListType.X, op=OP.max)
                xm = pool.tile([N_GRAPHS, D_OUT], f32)
                nc.vector.tensor_scalar(out=xm[:], in0=mean[:],
                                        scalar1=mx[:, 0:1], scalar2=None,
                                        op0=OP.subtract)
                ex = pool.tile([N_GRAPHS, D_OUT], f32)
                nc.scalar.activation(ex[:], xm[:], AF.Exp)
                ssum = pool.tile([N_GRAPHS, 1], f32)
                nc.vector.tensor_reduce(ssum[:], ex[:],
                                        axis=mybir.AxisListType.X, op=OP.add)
                lse = pool.tile([N_GRAPHS, 1], f32)
                nc.scalar.activation(lse[:], ssum[:], AF.Ln)
                fin = pool.tile([N_GRAPHS, D_OUT], f32)
                nc.vector.tensor_scalar(out=fin[:], in0=xm[:],
                                        scalar1=lse[:, 0:1], scalar2=None,
                                        op0=OP.subtract)
                nc.sync.dma_start(out=out_d.ap(), in_=fin[:])

    nc.compile()
    return nc


# --------------------------------------------------------------- entry point
_CACHE = {}


def prepare(inputs):
    x = np.asarray(inputs["x"], np.float32)
    edge_index = np.asarray(inputs["edge_index"])
    batch = np.asarray(inputs["batch"])
    W1 = np.asarray(inputs["W1"], np.float32)
    a1s = np.asarray(inputs["a1_src"], np.float32)
    a1d = np.asarray(inputs["a1_dst"], np.float32)
    b1 = np.asarray(inputs["b1"], np.float32)
    W2 = np.asarray(inputs["W2"], np.float32)
    a2s = np.asarray(inputs["a2_src"], np.float32)
    a2d = np.asarray(inputs["a2_dst"], np.float32)
    b2 = np.asarray(inputs["b2"], np.float32)

    meta, per_core = preprocess(edge_index, batch)
    with_b1 = bool(np.abs(b1).max() > 0)
    with_b2 = bool(np.abs(b2).max() > 0)

    key = (meta["TC"], with_b1, with_b2, meta["idx_lo_cols"],
           meta["idx_hi_cols"], tuple(int(v) for v in meta["CLO"]),
           tuple(int(v) for v in meta["CHI"]))
    if key not in _CACHE:
        _CACHE[key] = build_program(meta, with_b1, with_b2)
    nc = _CACHE[key]

    xpad = np.zeros((NPAD, P), np.float32)
    xpad[:N_NODES] = x
    t1 = np.zeros((NPAD, ELEM1), ml_dtypes.bfloat16)
    t1[:, 0:P] = xpad.astype(ml_dtypes.bfloat16)
    xTb = np.ascontiguousarray(xpad.T).astype(ml_dtypes.bfloat16)
    wad = np.stack([W1 @ a1s, W1 @ a1d], axis=1).astype(ml_dtypes.bfloat16)
    w1b = W1.astype(ml_dtypes.bfloat16)
    w2aug = np.concatenate(
        [W2, (W2 @ a2s)[:, None], (W2 @ a2d)[:, None]],
        axis=1).astype(ml_dtypes.bfloat16)
    iota = np.tile(np.arange(P, dtype=np.int16), (P, 1))
    cnt = np.bincount(batch.astype(np.int64), minlength=N_GRAPHS)
    rcnt = (1.0 / np.maximum(cnt, 1)).astype(np.float32)[:, None]

    in_maps = []
    for k in range(NCORES):
        m = dict(t1=t1, xTb=xTb, wad=wad, w1b=w1b, w2aug=w2aug,
                 idx_lo=per_core[k]["idx_lo"], idx_hi=per_core[k]["idx_hi"],
                 tileidx=per_core[k]["tileidx"],
                 dstoff=per_core[k]["dstoff"],
                 batchoff=per_core[k]["batchoff"],
                 iota=iota, recip_cnt=rcnt)
        if with_b1:
            m["b1col"] = b1[:, None].astype(np.float32)
        if with_b2:
            m["b2b"] = np.tile(b2[None, :], (P, 1)).astype(np.float32)
        in_maps.append(m)
    return nc, in_maps


def kernel(**inputs) -> np.ndarray:
    nc, in_maps = prepare(inputs)
    res = run_bass_kernel_spmd(nc, in_maps, list(range(NCORES)))
    return np.asarray(res.results[0]["out"], np.float32)
